# revision 1
# baseline (speedup 1.0000x reference)
"""Multi-head attention with RoPE on 8 Trainium2 NeuronCores.

Sharding: core c -> (batch g = c//4, head-group hg = c%4 of 4 heads).
Per core: QKV projection (column slice of w_qkv), RoPE (rotation via a
signed-permutation matmul + elementwise combine), attention computed as
S^T = K' Q'^T per 128-row j-tile (contraction d=64), exp on ScalarE
(no max-subtraction needed: scores are O(1) by construction), ones-column
appended to V so the softmax denominator falls out of the same PSUM
accumulation as P@V, per-row normalize, then the local out-projection
partial (rows of w_out for the local heads). Partials are summed across
each 4-core batch group with chunked ReduceScatter (one per 512-row
i-block, overlapped with compute); each core lands a distinct
[256-outcol x 512-row] quarter per chunk, and the host just transposes
and concatenates. Matmuls run as float32r (TF32-like) for 4x PE
throughput over fp32.
"""

import numpy as np

H, HD = 16, 64
B, N, DIM = 2, 2048, 1024
N_CORES = 8
GROUPS = [[0, 1, 2, 3], [4, 5, 6, 7]]

_COMPILED = {}


def _host_prep(x, w_qkv, w_out, b_out):
    freqs = 10000.0 ** (-np.arange(0, HD, 2, dtype=np.float32) / HD)
    angles = np.arange(N, dtype=np.float32)[:, None] * freqs
    sin = np.sin(angles).astype(np.float32)
    cos = np.cos(angles).astype(np.float32)
    sin_i = np.stack([sin, sin], axis=-1).reshape(N, HD)
    cos_i = np.stack([cos, cos], axis=-1).reshape(N, HD)
    cs = np.concatenate([cos_i.T, cos_i.T], 0).copy()  # [128, N]
    sn = np.concatenate([sin_i.T, sin_i.T], 0).copy()

    R = np.zeros((HD, HD), np.float32)
    for d in range(32):
        R[d, 2 * d + 1] = -1.0
    for d in range(32, 64):
        R[d, 2 * (d - 32)] = 1.0
    R2 = np.zeros((128, 128), np.float32)
    R2[:64, :64] = R
    R2[64:, 64:] = R
    r2t = np.ascontiguousarray(R2.T)

    in_maps = []
    for c in range(N_CORES):
        g, hg = c // 4, c % 4
        heads = range(4 * hg, 4 * hg + 4)
        w_qk = np.concatenate(
            [np.concatenate([w_qkv[:, h * 64:(h + 1) * 64],
                             w_qkv[:, DIM + h * 64: DIM + (h + 1) * 64]], axis=1)
             for h in heads], axis=1)
        w_v = np.concatenate(
            [w_qkv[:, 2 * DIM + h * 64: 2 * DIM + (h + 1) * 64] for h in heads], axis=1)
        w_o = np.ascontiguousarray(w_out[4 * hg * 64:(4 * hg + 4) * 64, :])
        b_o = np.ascontiguousarray((b_out / 4.0).reshape(8, 128).T)
        in_maps.append({
            "x_t": np.ascontiguousarray(x[g].T),
            "w_qk": np.ascontiguousarray(w_qk),
            "w_v": np.ascontiguousarray(w_v),
            "w_o": w_o,
            "b_o": b_o,
            "cs": cs,
            "sn": sn,
            "r2t": r2t,
            "ones": np.ones((128, 64), np.float32),
        })
    return in_maps


def build_nc(with_collective=True):
    import concourse.bass as bass  # noqa: F401
    import concourse.mybir as mybir
    import concourse.tile as tile
    from concourse import bacc

    f32 = mybir.dt.float32
    f32r = mybir.dt.float32r
    mult = mybir.AluOpType.mult
    add = mybir.AluOpType.add
    Exp = mybir.ActivationFunctionType.Exp

    nc = bacc.Bacc("TRN2", target_bir_lowering=False, debug=False,
                   num_devices=N_CORES)
    x_t = nc.dram_tensor("x_t", [DIM, N], f32r, kind="ExternalInput")
    w_qk = nc.dram_tensor("w_qk", [DIM, 512], f32r, kind="ExternalInput")
    w_v = nc.dram_tensor("w_v", [DIM, 256], f32r, kind="ExternalInput")
    w_o = nc.dram_tensor("w_o", [256, DIM], f32r, kind="ExternalInput")
    b_o = nc.dram_tensor("b_o", [128, 8], f32, kind="ExternalInput")
    cs_d = nc.dram_tensor("cs", [128, N], f32, kind="ExternalInput")
    sn_d = nc.dram_tensor("sn", [128, N], f32, kind="ExternalInput")
    r2t_d = nc.dram_tensor("r2t", [128, 128], f32r, kind="ExternalInput")
    ones_d = nc.dram_tensor("ones", [128, 64], f32r, kind="ExternalInput")
    y_out = nc.dram_tensor("y", [4, 256, 512], f32, kind="ExternalOutput")

    with tile.TileContext(nc) as tc:
        with (
            tc.tile_pool(name="persist", bufs=1) as persist,
            tc.tile_pool(name="ppS", bufs=2, space="PSUM") as ppS,
            tc.tile_pool(name="ppO", bufs=1, space="PSUM") as ppO,
            tc.tile_pool(name="ppC", bufs=2, space="PSUM") as ppC,
            tc.tile_pool(name="dram", bufs=8, space="DRAM") as dram,
        ):
            qp = persist.tile([64, 4, N], f32r)            # q'^T per head [d64, n]
            kp = persist.tile([64, 4, N], f32r)            # k'^T per head [d64, n]
            vsb = persist.tile([128, 16, 4, 65], f32r)     # v + ones col, per j-tile
            wo_sb = persist.tile([128, 2, DIM], f32r)
            b_sb = persist.tile([128, 8], f32)
            ones_sb = persist.tile([1, 64], f32r)

            def attn_jts(ihalf, h, ps_o, jts, epl, pre_jt=None):
                for jt in jts:
                    if pre_jt is not None:
                        pre_jt(jt)
                    ps_s = ppS.tile([128, 1024], f32, name="psA")
                    for half in range(2):
                        nc.tensor.matmul(
                            ps_s[:, half * 512:(half + 1) * 512],
                            lhsT=kp[:, h, jt * 128:(jt + 1) * 128],
                            rhs=qp[:, h,
                                   ihalf * 1024 + half * 512:
                                   ihalf * 1024 + (half + 1) * 512],
                            start=True, stop=True,
                        )
                    e_t = epl.tile([128, 1024], f32r, name="e_t")
                    nc.scalar.activation(e_t[:], ps_s[:], Exp, scale=0.125)
                    for half in range(2):
                        nc.tensor.matmul(
                            ps_o[0:65, half * 512:(half + 1) * 512],
                            lhsT=vsb[:, jt, h, :],
                            rhs=e_t[:, half * 512:(half + 1) * 512],
                            start=(jt == 0), stop=(jt == 15),
                        )

            # ---------------- Phase 1: QKV projection + RoPE ----------------
            with (
                tc.tile_pool(name="xw", bufs=1) as xw,
                tc.tile_pool(name="scr", bufs=4) as scr,
                tc.tile_pool(name="e0pool", bufs=3) as e0pool,
            ):
                cs_sb = xw.tile([128, N], f32)
                sn_sb = xw.tile([128, N], f32)
                r2t_sb = xw.tile([128, 128], f32r)
                wqk = xw.tile([128, 8, 512], f32r)
                wv = xw.tile([128, 8, 256], f32r)
                # compute-critical loads on the sync HWDGE FIFO: qk weights,
                # then the x chunks (issued in the ic4 loop right below).
                # Everything else rides the gpsimd SWDGE queues in parallel.
                xt0 = xw.tile([128, 8, 512], f32r, name="xt", bufs=2)
                for kt in range(8):
                    nc.gpsimd.dma_start(wqk[:, kt, :], w_qk[kt * 128:(kt + 1) * 128, :])
                for kt in range(8):
                    nc.sync.dma_start(
                        xt0[:, kt, :], x_t[kt * 128:(kt + 1) * 128, 0:512])
                nc.sync.dma_start(r2t_sb[:], r2t_d.ap())
                nc.gpsimd.dma_start(cs_sb[:], cs_d.ap())
                nc.gpsimd.dma_start(sn_sb[:], sn_d.ap())
                ones_stage = xw.tile([128, 64], f32r)
                nc.gpsimd.dma_start(ones_stage[:], ones_d[:, :])
                nc.gpsimd.dma_start(ones_sb[:], ones_d[0:1, :])
                nc.scalar.copy(
                    vsb[:, :, :, 64:65],
                    ones_stage[:, :].rearrange("p (a b c) -> p a b c", b=4, c=1))
                for kt in range(8):
                    nc.gpsimd.dma_start(wv[:, kt, :], w_v[kt * 128:(kt + 1) * 128, :])
                nc.gpsimd.dma_start(b_sb[:], b_o.ap())
                for kt in range(2):
                    nc.gpsimd.dma_start(wo_sb[:, kt, :], w_o[kt * 128:(kt + 1) * 128, :])

                for ic4 in range(4):
                    isl = slice(ic4 * 512, (ic4 + 1) * 512)
                    if ic4 == 0:
                        xt = xt0
                    else:
                        xt = xw.tile([128, 8, 512], f32r, name="xt", bufs=2)
                        for kt in range(8):
                            nc.sync.dma_start(xt[:, kt, :], x_t[kt * 128:(kt + 1) * 128, isl])
                    # rope chain for head h-1 emitted after head h's qk matmuls
                    # so the rot matmul never heads the PE queue waiting on its
                    # ACT copy round trip
                    pend = None

                    def rope_chain(h, qks):
                        ps_rot = ppC.tile([128, 512], f32, name="psC")
                        nc.tensor.matmul(ps_rot[:, 0:512], lhsT=r2t_sb[:],
                                         rhs=qks[:], start=True, stop=True)
                        t1 = scr.tile([128, 512], f32, name="t1")
                        nc.vector.tensor_tensor(t1[:], qks[:].bitcast(f32), cs_sb[:, isl], op=mult)
                        t2 = scr.tile([128, 512], f32, name="t2")
                        nc.vector.tensor_tensor(t2[:], ps_rot[:, 0:512], sn_sb[:, isl], op=mult)
                        nc.vector.tensor_tensor(qp[:, h, isl], t1[0:64, :], t2[0:64, :], op=add)
                        nc.vector.tensor_tensor(kp[:, h, isl], t1[64:128, :], t2[64:128, :], op=add)

                    for h in range(4):
                        ps_qk = ppS.tile([128, 1024], f32, name="psA")
                        for kt in range(8):
                            nc.tensor.matmul(
                                ps_qk[:, 0:512],
                                lhsT=wqk[:, kt, h * 128:(h + 1) * 128],
                                rhs=xt[:, kt, :],
                                start=(kt == 0), stop=(kt == 7),
                            )
                        qks = scr.tile([128, 512], f32r, name="qks")
                        nc.scalar.copy(qks[:], ps_qk[:, 0:512])
                        if pend is not None:
                            rope_chain(*pend)
                        pend = (h, qks)
                    rope_chain(*pend)
                    for it2 in range(4):
                        it = ic4 * 4 + it2
                        ps_v = ppC.tile([128, 512], f32, name="psC")
                        for kt in range(8):
                            nc.tensor.matmul(
                                ps_v[:, 0:256],
                                lhsT=xt[:, kt, it2 * 128:(it2 + 1) * 128],
                                rhs=wv[:, kt, :],
                                start=(kt == 0), stop=(kt == 7),
                            )
                        nc.vector.tensor_copy(
                            vsb[:, it, :, 0:64],
                            ps_v[:, 0:256].rearrange("p (h d) -> p h d", d=64),
                        )
                    if ic4 == 1:
                        ps_o0 = ppO.tile([128, 1024], f32, name="psO")
                        attn_jts(0, 0, ps_o0, range(0, 8), e0pool)
                    elif ic4 == 2:
                        attn_jts(0, 0, ps_o0, range(8, 12), e0pool)
                    elif ic4 == 3:
                        attn_jts(0, 0, ps_o0, range(12, 16), e0pool)

            # ---------------- Phase 2+3: attention, out-proj, RS ----------------
            with (
                tc.tile_pool(name="epool", bufs=8) as epool,
                tc.tile_pool(name="opool", bufs=1) as opool,
                tc.tile_pool(name="npool", bufs=4) as npool,
                tc.tile_pool(name="outp", bufs=8) as outp,
            ):
                osb_all = {}

                def attn_norm(ihalf, h, ps_o):
                    # reciprocal computed in halves so the first broadcast
                    # matmul (at the in-order PE queue head) unblocks after
                    # ~0.6us instead of the full-width reciprocal latency
                    osb = osb_all[ihalf]
                    recip = npool.tile([1, 1024], f32r, name="recip")
                    bc_sb = npool.tile([64, 1024], f32, name="bc_sb")
                    for half in range(2):
                        hs = slice(half * 512, (half + 1) * 512)
                        with nc.allow_low_precision(reason="recip feeds f32r matmul"):
                            nc.vector.reciprocal(recip[:, hs], ps_o[64:65, hs])
                        ps_b = ppC.tile([128, 512], f32, name="psC")
                        nc.tensor.matmul(
                            ps_b[0:64, :],
                            lhsT=ones_sb[:],
                            rhs=recip[:, hs],
                            start=True, stop=True,
                        )
                        nc.vector.tensor_copy(bc_sb[:, hs], ps_b[0:64, :])
                    nc.vector.tensor_tensor(
                        osb[h // 2][(h % 2) * 64:(h % 2) * 64 + 64, :],
                        ps_o[0:64, :], bc_sb[:], op=mult)

                def attn_head(ihalf, h):
                    ps_o = ppO.tile([128, 1024], f32, name="psO")
                    attn_jts(ihalf, h, ps_o, range(16), epool)
                    attn_norm(ihalf, h, ps_o)

                def outproj_oc(ihalf, half, oc, rs_in):
                    osb = osb_all[ihalf]
                    ps_out = ppC.tile([128, 512], f32, name="psC")
                    for kt in range(2):
                        nc.tensor.matmul(
                            ps_out[:, :],
                            lhsT=wo_sb[:, kt, oc * 128:(oc + 1) * 128],
                            rhs=osb[kt][:, half * 512:(half + 1) * 512],
                            start=(kt == 0), stop=(kt == 1),
                        )
                    o_t = outp.tile([128, 512], f32, name="o_t")
                    nc.vector.tensor_scalar_add(o_t[:], ps_out[:, :],
                                                b_sb[:, oc:oc + 1])
                    nc.sync.dma_start(rs_in[oc * 128:(oc + 1) * 128, :], o_t[:])

                def rs_fire(ib, rs_in):
                    if with_collective:
                        rs_out = dram.tile([256, 512], f32, name=f"rs_out_{ib}")
                        nc.gpsimd.collective_compute(
                            "ReduceScatter",
                            mybir.AluOpType.add,
                            replica_groups=GROUPS,
                            ins=[rs_in[:]],
                            outs=[rs_out[:]],
                        )
                        nc.sync.dma_start(y_out[ib], rs_out[:])
                    else:
                        nc.sync.dma_start(y_out[ib], rs_in[0:256, :])

                def attn_head_carrying(ihalf, h, co_ihalf, co_half):
                    # run a head's attention with the previous i-half's
                    # out-projection spread one oc per 2 j-tiles, filling the
                    # ACT-bound per-jt PE slack instead of bursting 16 matmuls
                    ib = 2 * co_ihalf + co_half
                    rs_in = dram.tile([1024, 512], f32, name=f"rs_in_{ib}")

                    def pre(jt):
                        if jt % 2 == 1:
                            outproj_oc(co_ihalf, co_half, jt // 2, rs_in)
                    ps_o = ppO.tile([128, 1024], f32, name="psO")
                    attn_jts(ihalf, h, ps_o, range(16), epool, pre)
                    attn_norm(ihalf, h, ps_o)
                    rs_fire(ib, rs_in)

                def outproj_rs(ihalf, half):
                    ib = 2 * ihalf + half
                    rs_in = dram.tile([1024, 512], f32, name=f"rs_in_{ib}")
                    for oc in range(8):
                        outproj_oc(ihalf, half, oc, rs_in)
                    if with_collective:
                        rs_out = dram.tile([256, 512], f32, name=f"rs_out_{ib}")
                        nc.gpsimd.collective_compute(
                            "ReduceScatter",
                            mybir.AluOpType.add,
                            replica_groups=GROUPS,
                            ins=[rs_in[:]],
                            outs=[rs_out[:]],
                        )
                        nc.sync.dma_start(y_out[ib], rs_out[:])
                    else:
                        nc.sync.dma_start(y_out[ib], rs_in[0:256, :])

                # interleave ihalf=0's out-projection between ihalf=1's heads so
                # its PSUM slots and DVE drains overlap ACT-bound attention
                osb_all[0] = [opool.tile([128, 1024], f32r, name=f"osb0_{kt}") for kt in range(2)]
                osb_all[1] = [opool.tile([128, 1024], f32r, name=f"osb1_{kt}") for kt in range(2)]
                attn_norm(0, 0, ps_o0)  # j-loop ran interleaved with phase 1
                for h in range(1, 4):
                    attn_head(0, h)
                attn_head(1, 0)
                attn_head_carrying(1, 1, 0, 0)
                attn_head_carrying(1, 2, 0, 1)
                attn_head(1, 3)
                outproj_rs(1, 0)
                outproj_rs(1, 1)

    nc.compile()
    return nc


def _get_nc():
    if "nc" not in _COMPILED:
        _COMPILED["nc"] = build_nc()
    return _COMPILED["nc"]


def kernel(x, w_qkv, w_out, b_out):
    from concourse import bass_utils

    x = np.asarray(x, dtype=np.float32)
    w_qkv = np.asarray(w_qkv, dtype=np.float32)
    w_out = np.asarray(w_out, dtype=np.float32)
    b_out = np.asarray(b_out, dtype=np.float32)

    nc = _get_nc()
    in_maps = _host_prep(x, w_qkv, w_out, b_out)
    res = bass_utils.run_bass_kernel_spmd(nc, in_maps, list(range(N_CORES)))

    out = np.zeros((B, N, DIM), np.float32)
    for c in range(N_CORES):
        g, pos = c // 4, c % 4
        y = res.results[c]["y"]  # [4, 256, 512]
        for ib in range(4):
            out[g, ib * 512:(ib + 1) * 512, pos * 256:(pos + 1) * 256] = y[ib].T
    return out


if __name__ == "__main__":
    rng = np.random.default_rng(0)
    x = rng.standard_normal((B, N, DIM)).astype(np.float32)
    w_qkv = (rng.standard_normal((DIM, 3 * DIM)) * DIM ** -0.5).astype(np.float32)
    w_out = (rng.standard_normal((DIM, DIM)) * DIM ** -0.5).astype(np.float32)
    b_out = np.zeros(DIM, np.float32)
    out = kernel(x, w_qkv, w_out, b_out)
    print("out", out.shape, out.dtype, float(np.abs(out).max()))



# revision 39
# speedup vs baseline: 1.2659x; 1.2659x over previous
"""Multi-head attention with RoPE on 8 Trainium2 NeuronCores.

Sharding: core c -> (batch g = c//4, head-group hg = c%4 of 4 heads).

Per core, all-bf16 dataflow (PSUM accumulation in f32):
  - QKV projection from column slices of w_qkv; RoPE via a signed-permutation
    matmul + DVE combines, q'/k' stored dim-major [64, h, n] in bf16.
  - Attention runs per (i-quarter, head) pair: scores S^T = K'_jt Q'_iq as
    [128 j x 512 i] PSUM tiles, exp on ACT (no max-subtraction: scores O(1)
    by construction) into bf16 e_t tiles.
  - P@V uses the swapped operand order: lhsT = e_t i-slice (stationary),
    rhs = V j-tile with a ones column appended, so the output lands [i, d+1]
    with full 128-partition utilization (65 PE rows per j-tile instead of
    512) and the softmax denominator in column 64.
  - Normalize on DVE via per-partition reciprocal * tensor_scalar, then the
    [i, d] -> [d, i] flip runs on the DMA crossbar transpose engine (free on
    PE), feeding the out-projection (rows of w_out for the local heads).
  - Partials summed across each 4-core batch group with chunked
    ReduceScatter (one per 512-row i-quarter, overlapped with compute);
    each core lands a distinct [256-outcol x 512-row] quarter per chunk.

The emission schedule pipelines pairs: during pair P's score/exp stream the
PE also runs pair P-1's P@V, the previous i-quarter's transposes and the
out-projection + ReduceScatter of the quarter before that, keeping ACT (the
exp engine) saturated while PE stays ahead of it.
"""

import numpy as np

H, HD = 16, 64
B, N, DIM = 2, 2048, 1024
N_CORES = 8
GROUPS = [[0, 1, 2, 3], [4, 5, 6, 7]]

NJT = 16          # 128-token j tiles
NIQ = 4           # 512-token i quarters (== ReduceScatter chunks)
NIT = 4           # 128-token i tiles per quarter
NPAIR = NIQ * 4   # (i-quarter, head) pairs per core

_COMPILED = {}


def _host_prep(x, w_qkv, w_out, b_out):
    import ml_dtypes

    bf = ml_dtypes.bfloat16
    freqs = 10000.0 ** (-np.arange(0, HD, 2, dtype=np.float32) / HD)
    angles = np.arange(N, dtype=np.float32)[:, None] * freqs
    sin = np.sin(angles).astype(np.float32)
    cos = np.cos(angles).astype(np.float32)
    sin_i = np.stack([sin, sin], axis=-1).reshape(N, HD)
    cos_i = np.stack([cos, cos], axis=-1).reshape(N, HD)
    cs = np.concatenate([cos_i.T, cos_i.T], 0).astype(bf)  # [128, N]
    sn = np.concatenate([sin_i.T, sin_i.T], 0).astype(bf)

    R = np.zeros((HD, HD), np.float32)
    for d in range(32):
        R[d, 2 * d + 1] = -1.0
    for d in range(32, 64):
        R[d, 2 * (d - 32)] = 1.0
    R2 = np.zeros((128, 128), np.float32)
    R2[:64, :64] = R
    R2[64:, 64:] = R
    r2t = np.ascontiguousarray(R2.T).astype(bf)

    in_maps = []
    for c in range(N_CORES):
        g, hg = c // 4, c % 4
        heads = range(4 * hg, 4 * hg + 4)
        w_qk = np.concatenate(
            [np.concatenate([w_qkv[:, h * 64:(h + 1) * 64],
                             w_qkv[:, DIM + h * 64: DIM + (h + 1) * 64]], axis=1)
             for h in heads], axis=1)
        w_v = np.concatenate(
            [w_qkv[:, 2 * DIM + h * 64: 2 * DIM + (h + 1) * 64] for h in heads], axis=1)
        w_o = np.ascontiguousarray(w_out[4 * hg * 64:(4 * hg + 4) * 64, :])
        b_o = np.ascontiguousarray((b_out / 4.0).reshape(8, 128).T)
        in_maps.append({
            "x_t": np.ascontiguousarray(x[g].T).astype(bf),
            "w_qk": np.ascontiguousarray(w_qk).astype(bf),
            "w_v": np.ascontiguousarray(w_v).astype(bf),
            "w_o": w_o.astype(bf),
            "b_o": b_o.astype(np.float32),
            "cs": np.ascontiguousarray(cs),
            "sn": np.ascontiguousarray(sn),
            "r2t": r2t,
            "id128": np.eye(128, dtype=bf),
        })
    return in_maps


def build_nc(with_collective=True, dve_groups=(1, 3, 4, 6), batch_w=True):
    import concourse.bass as bass  # noqa: F401
    import concourse.mybir as mybir
    import concourse.tile as tile
    from concourse import bacc

    f32 = mybir.dt.float32
    bf16 = mybir.dt.bfloat16
    i16 = mybir.dt.int16
    mult = mybir.AluOpType.mult
    add = mybir.AluOpType.add
    Exp = mybir.ActivationFunctionType.Exp

    # Schraudolph-style exp for the DVE offload path: e^(x/8) computed as
    # bf16 bit pattern round(x * 16*log2(e) + (127*128 - c)) written as
    # int16 and bitcast to bf16. c tuned for round-to-nearest; max rel err
    # ~3% which the softmax tolerates (weights err averages out across j).
    SCH_A = 128.0 * 1.4426950408889634 * 0.125
    SCH_B = 16256.0 - 5.7

    nc = bacc.Bacc("TRN2", target_bir_lowering=False, debug=False,
                   num_devices=N_CORES)
    x_t = nc.dram_tensor("x_t", [DIM, N], bf16, kind="ExternalInput")
    w_qk = nc.dram_tensor("w_qk", [DIM, 512], bf16, kind="ExternalInput")
    w_v = nc.dram_tensor("w_v", [DIM, 256], bf16, kind="ExternalInput")
    w_o = nc.dram_tensor("w_o", [256, DIM], bf16, kind="ExternalInput")
    b_o = nc.dram_tensor("b_o", [128, 8], f32, kind="ExternalInput")
    cs_d = nc.dram_tensor("cs", [128, N], bf16, kind="ExternalInput")
    sn_d = nc.dram_tensor("sn", [128, N], bf16, kind="ExternalInput")
    r2t_d = nc.dram_tensor("r2t", [128, 128], bf16, kind="ExternalInput")
    id_d = nc.dram_tensor("id128", [128, 128], bf16, kind="ExternalInput")
    y_out = nc.dram_tensor("y", [4, 256, 512], f32, kind="ExternalOutput")

    with tile.TileContext(nc) as tc:
        with (
            tc.tile_pool(name="persist", bufs=1) as persist,
            tc.tile_pool(name="epool", bufs=5) as epool,
            tc.tile_pool(name="npool", bufs=2) as npool,
            tc.tile_pool(name="rpool", bufs=4) as rpool,
            tc.tile_pool(name="outp", bufs=4) as outp,
            tc.tile_pool(name="ppS", bufs=3, space="PSUM") as ppS,
            tc.tile_pool(name="ppA", bufs=2, space="PSUM") as ppA,
            tc.tile_pool(name="dram", bufs=8, space="DRAM") as dram,
        ):
            qp = persist.tile([64, 4, N], bf16)            # q'^T per head [d64, n]
            kp = persist.tile([64, 4, N], bf16)            # k'^T per head [d64, n]
            vsb = persist.tile([128, NJT, 4, 65], bf16)    # v + ones col per j-tile
            wo_sb = persist.tile([128, 2, DIM], bf16)
            b_sb = persist.tile([128, 8], f32)
            osb = persist.tile([128, 2, N], bf16)          # attn out, [d-kt, i]
            id_sb = persist.tile([128, 128], bf16)

            # ---- pair bookkeeping ---------------------------------------
            e_tiles = {}          # pair -> e_t tile [128, NJT, 512]
            emitted = {}          # pair -> set of emitted score groups
            nsb_tiles = {}        # iq -> [128, NIT, 256] bf16
            rs_tiles = {}         # iq -> rs_in dram tile

            def pq(P):
                return P // 4, P % 4  # (iq, h)

            # DVE-offloaded exp groups sit at the END of each pair so the ACT
            # stream stays contiguous: the next pair's scores recycle the
            # ps_s slots these free (DVE drains them fast), never stalling
            # behind a pending ACT exp.
            DVE_EXP_GROUPS = dve_groups

            def emit_scores_group(P, g):
                """Scores + exp for j-tiles 2g, 2g+1 of pair P."""
                if g in emitted.setdefault(P, set()):
                    return
                emitted[P].add(g)
                iq, h = pq(P)
                if P not in e_tiles:
                    e_tiles[P] = epool.tile([128, NJT, 512], bf16, name="e_t")
                ps = ppS.tile([128, 1024], f32, name="psS")
                for t in range(2):
                    jt = 2 * g + t
                    nc.tensor.matmul(
                        ps[:, t * 512:(t + 1) * 512],
                        lhsT=kp[:, h, jt * 128:(jt + 1) * 128],
                        rhs=qp[:, h, iq * 512:(iq + 1) * 512],
                        start=True, stop=True,
                    )
                e_out = e_tiles[P][:, 2 * g:2 * g + 2, :]
                # head-3 pairs offload EARLY groups instead: their late exps
                # gate the next quarter's P@V + transpose chain
                dve_set = (0, 2) if h == 3 else DVE_EXP_GROUPS
                if P >= 4 and g in dve_set:
                    # approximate exp on the otherwise-idle DVE to unload ACT
                    with nc.allow_low_precision(reason="schraudolph exp"):
                        nc.vector.tensor_scalar(
                            e_out.bitcast(i16), ps[:], SCH_A, SCH_B,
                            op0=mult, op1=add)
                else:
                    nc.scalar.activation(e_out, ps[:], Exp, scale=0.125)

            pv_ps = {}  # (P, it) -> psum accum tile
            pv_emitted = set()

            def emit_pv_half(P, it, hi):
                """8 accumulation matmuls for i-tile `it` of pair P over the
                low or high 8 j-tiles."""
                if (P, it, hi) in pv_emitted:
                    return
                pv_emitted.add((P, it, hi))
                iq, h = pq(P)
                e_t = e_tiles[P]
                if not hi:
                    pv_ps[(P, it)] = ppA.tile([128, 512], f32, name="psA")
                ps = pv_ps[(P, it)]
                for jj in range(8):
                    jt = 8 * hi + jj
                    nc.tensor.matmul(
                        ps[:, 0:65],
                        lhsT=e_t[:, jt, it * 128:(it + 1) * 128],
                        rhs=vsb[:, jt, h, :],
                        start=(jt == 0), stop=(jt == 15),
                    )

            def emit_normalize(P, it):
                iq, h = pq(P)
                ps = pv_ps.pop((P, it))
                if iq not in nsb_tiles:
                    nsb_tiles[iq] = npool.tile([128, NIT, 256], bf16, name="nsb")
                recip = rpool.tile([128, 1], f32, name="recip")
                with nc.allow_low_precision(reason="softmax denom recip"):
                    nc.vector.reciprocal(recip[:], ps[:, 64:65])
                nc.vector.tensor_scalar_mul(
                    nsb_tiles[iq][:, it, h * 64:(h + 1) * 64],
                    ps[:, 0:64], recip[:, 0:1])

            def emit_transpose(iq, it):
                # [128 i, 256 d] -> osb[:, kt, i] via DMA crossbar transpose
                nc.sync.dma_start_transpose(
                    out=osb[:, :, iq * 512 + it * 128: iq * 512 + (it + 1) * 128],
                    in_=nsb_tiles[iq][:, it, :],
                )

            def emit_transpose_pe(iq, it):
                # latency-critical tail variant: PE transpose + DVE copy
                # (~0.5us) instead of the ~2.5us DMA crossbar round trip
                ps_t = ppS.tile([128, 256], bf16, name="psS")
                for kt in range(2):
                    nc.tensor.transpose(
                        ps_t[:, kt * 128:(kt + 1) * 128],
                        nsb_tiles[iq][:, it, kt * 128:(kt + 1) * 128],
                        id_sb[:])
                nc.vector.tensor_copy(
                    osb[:, :, iq * 512 + it * 128: iq * 512 + (it + 1) * 128],
                    ps_t[:].rearrange("p (kt m) -> p kt m", kt=2))

            def emit_outproj_oc(iq, oc):
                if iq not in rs_tiles:
                    rs_tiles[iq] = dram.tile([1024, 512], f32, name=f"rs_in_{iq}")
                ps = ppS.tile([128, 512], f32, name="psS")
                for kt in range(2):
                    nc.tensor.matmul(
                        ps[:, :],
                        lhsT=wo_sb[:, kt, oc * 128:(oc + 1) * 128],
                        rhs=osb[:, kt, iq * 512:(iq + 1) * 512],
                        start=(kt == 0), stop=(kt == 1),
                    )
                o_t = outp.tile([128, 512], f32, name="o_t")
                # PSUM is only reachable from DVE/ACT on real hardware, and
                # ACT's Copy path cannot take a per-partition bias AP
                nc.vector.tensor_scalar_add(o_t[:], ps[:, :], b_sb[:, oc:oc + 1])
                nc.sync.dma_start(rs_tiles[iq][oc * 128:(oc + 1) * 128, :], o_t[:])

            def rs_fire(iq):
                rs_in = rs_tiles[iq]
                if with_collective:
                    rs_out = dram.tile([256, 512], f32, name=f"rs_out_{iq}")
                    nc.gpsimd.collective_compute(
                        "ReduceScatter",
                        mybir.AluOpType.add,
                        replica_groups=GROUPS,
                        ins=[rs_in[:]],
                        outs=[rs_out[:]],
                    )
                    nc.sync.dma_start(y_out[iq], rs_out[:])
                else:
                    nc.sync.dma_start(y_out[iq], rs_in[0:256, :])

            # PV emission order within a drive: (it, hi) per g slot
            PV_ORDER = [(0, 0), (1, 0), (0, 1), (1, 1), (2, 0), (3, 0), (2, 1), (3, 1)]

            def drive(P):
                """Emit one pipeline step: pair P+1's remaining scores
                interleaved with pair P-1's P@V, plus carried transpose /
                out-projection / ReduceScatter work."""
                iq, h = pq(P)
                for g in range(8):
                    if P - 1 >= 0:
                        it, hi = PV_ORDER[g]
                        emit_pv_half(P - 1, it, hi)
                        if hi:
                            emit_normalize(P - 1, it)
                            iqp, hp = pq(P - 1)
                            if hp == 3:
                                emit_transpose(iqp, it)
                                if it == NIT - 1:
                                    # e_t slot of P-1 and nsb of iqp retire here
                                    del e_tiles[P - 1]
                                    del nsb_tiles[iqp]
                    if 0 <= P + 1 < NPAIR:
                        emit_scores_group(P + 1, g)
                    elif P == NPAIR - 1 and g >= 4:
                        # last drive has no next-pair scores: pull the final
                        # pair's low-half P@V forward out of the tail
                        emit_pv_half(P, g - 4, 0)
                    # out-projection of quarter iq-1 spread across two drives
                    if iq >= 1 and h in (1, 2) and g % 2 == 1:
                        emit_outproj_oc(iq - 1, 4 * (h - 1) + g // 2)
                        if h == 2 and g == 7:
                            rs_fire(iq - 1)

            # ---------------- Phase 1: QKV projection + RoPE ----------------
            with (
                tc.tile_pool(name="xw", bufs=1) as xw,
                tc.tile_pool(name="scr", bufs=4) as scr,
            ):
                cs_sb = xw.tile([128, N], bf16)
                sn_sb = xw.tile([128, N], bf16)
                r2t_sb = xw.tile([128, 128], bf16)
                wqk = xw.tile([128, 8, 512], bf16)
                wv = xw.tile([128, 8, 256], bf16)
                xt0 = xw.tile([128, 8, 512], bf16, name="xt", bufs=2)
                # first compute chunk (wqk/x kt=0) leads both queues so the
                # first qk matmul starts as early as possible; remaining
                # weights ride single multi-descriptor DMAs so the SWDGE
                # queue's ~1us per-instruction generation cost doesn't starve
                # the rope (cs/sn) and v-projection (wv) mid-phase-1
                nc.gpsimd.dma_start(wqk[:, 0, :], w_qk[0:128, :])
                nc.sync.dma_start(xt0[:, 0, :], x_t[0:128, 0:512])
                if batch_w:
                    nc.gpsimd.dma_start(
                        wqk[:, 1:4, :],
                        w_qk[128:512, :].rearrange("(kt p) d -> p kt d", p=128))
                    nc.gpsimd.dma_start(
                        wqk[:, 4:8, :],
                        w_qk[512:1024, :].rearrange("(kt p) d -> p kt d", p=128))
                    nc.gpsimd.dma_start(
                        wv[:, :, :],
                        w_v.ap().rearrange("(kt p) d -> p kt d", p=128))
                else:
                    for kt in range(1, 8):
                        nc.gpsimd.dma_start(wqk[:, kt, :], w_qk[kt * 128:(kt + 1) * 128, :])
                    for kt in range(8):
                        nc.gpsimd.dma_start(wv[:, kt, :], w_v[kt * 128:(kt + 1) * 128, :])
                for kt in range(1, 8):
                    nc.sync.dma_start(
                        xt0[:, kt, :], x_t[kt * 128:(kt + 1) * 128, 0:512])
                nc.sync.dma_start(r2t_sb[:], r2t_d.ap())
                nc.sync.dma_start(cs_sb[:], cs_d.ap())
                nc.sync.dma_start(sn_sb[:], sn_d.ap())
                nc.gpsimd.dma_start(id_sb[:], id_d.ap())
                nc.vector.memset(vsb[:, :, :, 64:65], 1.0)
                nc.gpsimd.dma_start(b_sb[:], b_o.ap())
                nc.gpsimd.dma_start(
                    wo_sb[:, :, :],
                    w_o.ap().rearrange("(kt p) d -> p kt d", p=128))

                for ic4 in range(4):
                    # early score/exp groups interleaved at 8 points per ic4
                    # block (after each head's rope and each v-proj) so ACT
                    # stays busy without serializing the PE on the ps_s ring:
                    # ic4 c emits groups 2(c-1), 2(c-1)+1 for pairs 0..3,
                    # whose j-tiles were produced by block c-1.
                    if ic4 >= 1:
                        gs = (2 * (ic4 - 1), 2 * (ic4 - 1) + 1)
                        pend_scores = [(Pp, g) for g in gs for Pp in (0, 1, 2)]
                        pend_scores += [(3, g) for g in gs]
                    else:
                        pend_scores = []

                    def pump_scores():
                        if pend_scores:
                            emit_scores_group(*pend_scores.pop(0))

                    isl = slice(ic4 * 512, (ic4 + 1) * 512)
                    if ic4 == 0:
                        xt = xt0
                    else:
                        xt = xw.tile([128, 8, 512], bf16, name="xt", bufs=2)
                        for kt in range(8):
                            nc.sync.dma_start(xt[:, kt, :], x_t[kt * 128:(kt + 1) * 128, isl])
                    # rope chain for head h-1 emitted after head h's qk matmuls
                    # so the rot matmul never heads the PE queue waiting on its
                    # DVE copy round trip
                    pend = None

                    def rope_chain(h, qks):
                        ps_rot = ppS.tile([128, 512], f32, name="psS")
                        nc.tensor.matmul(ps_rot[:, 0:512], lhsT=r2t_sb[:],
                                         rhs=qks[:], start=True, stop=True)
                        t1 = scr.tile([128, 512], bf16, name="t1")
                        nc.vector.tensor_tensor(t1[:], qks[:], cs_sb[:, isl], op=mult)
                        t2 = scr.tile([128, 512], bf16, name="t2")
                        nc.vector.tensor_tensor(t2[:], ps_rot[:, 0:512], sn_sb[:, isl], op=mult)
                        nc.vector.tensor_tensor(qp[:, h, isl], t1[0:64, :], t2[0:64, :], op=add)
                        nc.vector.tensor_tensor(kp[:, h, isl], t1[64:128, :], t2[64:128, :], op=add)

                    for h in range(4):
                        ps_qk = ppA.tile([128, 512], f32, name="psA")
                        for kt in range(8):
                            nc.tensor.matmul(
                                ps_qk[:, 0:512],
                                lhsT=wqk[:, kt, h * 128:(h + 1) * 128],
                                rhs=xt[:, kt, :],
                                start=(kt == 0), stop=(kt == 7),
                            )
                        qks = scr.tile([128, 512], bf16, name="qks")
                        nc.vector.tensor_copy(qks[:], ps_qk[:, 0:512])
                        if pend is not None:
                            rope_chain(*pend)
                            pump_scores()
                        pend = (h, qks)
                    rope_chain(*pend)
                    pump_scores()
                    for it2 in range(4):
                        it = ic4 * 4 + it2
                        ps_v = ppA.tile([128, 512], f32, name="psA")
                        for kt in range(8):
                            nc.tensor.matmul(
                                ps_v[:, 0:256],
                                lhsT=xt[:, kt, it2 * 128:(it2 + 1) * 128],
                                rhs=wv[:, kt, :],
                                start=(kt == 0), stop=(kt == 7),
                            )
                        nc.vector.tensor_copy(
                            vsb[:, it, :, 0:64],
                            ps_v[:, 0:256].rearrange("p (h d) -> p h d", d=64),
                        )
                        pump_scores()
                    while pend_scores:
                        pump_scores()

            # ---------------- Phase 2: pipelined attention ----------------
            # phase 1 covered groups 0..5 for pairs 0..3; pair 0's last two
            # groups have no preceding drive, so emit them first
            for g in range(6, 8):
                emit_scores_group(0, g)
            for P in range(NPAIR):
                drive(P)
            # tail: PV of the last pair + low-latency PE transposes, then the
            # final out-projection + ReduceScatter chunk
            for g in range(8):
                it, hi = PV_ORDER[g]
                emit_pv_half(NPAIR - 1, it, hi)
                if hi:
                    emit_normalize(NPAIR - 1, it)
                    emit_transpose_pe(NIQ - 1, it)
            for oc in range(8):
                emit_outproj_oc(NIQ - 1, oc)
            rs_fire(NIQ - 1)

    nc.compile()
    return nc


def _get_nc():
    if "nc" not in _COMPILED:
        _COMPILED["nc"] = build_nc()
    return _COMPILED["nc"]


def kernel(x, w_qkv, w_out, b_out):
    from concourse import bass_utils

    x = np.asarray(x, dtype=np.float32)
    w_qkv = np.asarray(w_qkv, dtype=np.float32)
    w_out = np.asarray(w_out, dtype=np.float32)
    b_out = np.asarray(b_out, dtype=np.float32)

    nc = _get_nc()
    in_maps = _host_prep(x, w_qkv, w_out, b_out)
    res = bass_utils.run_bass_kernel_spmd(nc, in_maps, list(range(N_CORES)))

    out = np.zeros((B, N, DIM), np.float32)
    for c in range(N_CORES):
        g, pos = c // 4, c % 4
        y = res.results[c]["y"]  # [4, 256, 512]
        for ib in range(4):
            out[g, ib * 512:(ib + 1) * 512, pos * 256:(pos + 1) * 256] = y[ib].T
    return out


if __name__ == "__main__":
    rng = np.random.default_rng(0)
    x = rng.standard_normal((B, N, DIM)).astype(np.float32)
    w_qkv = (rng.standard_normal((DIM, 3 * DIM)) * DIM ** -0.5).astype(np.float32)
    w_out = (rng.standard_normal((DIM, DIM)) * DIM ** -0.5).astype(np.float32)
    b_out = np.zeros(DIM, np.float32)
    out = kernel(x, w_qkv, w_out, b_out)
    print("out", out.shape, out.dtype, float(np.abs(out).max()))


# revision 70
# speedup vs baseline: 1.3952x; 1.1021x over previous
"""Multi-head attention with RoPE on 8 Trainium2 NeuronCores.

Sharding: core c -> (batch g = c//4, head-group hg = c%4 of 4 heads).

Per core, all-bf16 dataflow (PSUM accumulation in f32):
  - QKV projection from column slices of w_qkv; RoPE via a signed-permutation
    matmul + DVE combines, q'/k' stored dim-major [64, h, n] in bf16.
  - Attention runs per (i-quarter, head) pair: scores S^T = K'_jt Q'_iq as
    [128 j x 512 i] PSUM tiles, exp on ACT (no max-subtraction: scores O(1)
    by construction) into bf16 e_t tiles.
  - P@V uses the swapped operand order: lhsT = e_t i-slice (stationary),
    rhs = V j-tile with a ones column appended, so the output lands [i, d+1]
    with full 128-partition utilization (65 PE rows per j-tile instead of
    512) and the softmax denominator in column 64.
  - Normalize on DVE via per-partition reciprocal * tensor_scalar, then the
    [i, d] -> [d, i] flip runs on the DMA crossbar transpose engine (free on
    PE), feeding the out-projection (rows of w_out for the local heads).
  - Partials summed across each 4-core batch group with chunked
    ReduceScatter (one per 512-row i-quarter, overlapped with compute);
    each core lands a distinct [256-outcol x 512-row] quarter per chunk.

The emission schedule pipelines pairs: during pair P's score/exp stream the
PE also runs pair P-1's P@V, the previous i-quarter's transposes and the
out-projection + ReduceScatter of the quarter before that. Roughly a third
of the exp groups run on the otherwise-idle DVE via a Schraudolph-style
approximate exp (tensor_scalar to int16 bf16-bit-pattern), keeping ACT (the
exp engine, the secondary bottleneck after PE) off the critical path.
"""

import numpy as np

H, HD = 16, 64
B, N, DIM = 2, 2048, 1024
N_CORES = 8
GROUPS = [[0, 1, 2, 3], [4, 5, 6, 7]]

NJT = 16          # 128-token j tiles
NIQ = 4           # 512-token i quarters (== ReduceScatter chunks)
NIT = 4           # 128-token i tiles per quarter
NPAIR = NIQ * 4   # (i-quarter, head) pairs per core

_COMPILED = {}


def _host_prep(x, w_qkv, w_out, b_out):
    import ml_dtypes

    bf = ml_dtypes.bfloat16
    freqs = 10000.0 ** (-np.arange(0, HD, 2, dtype=np.float32) / HD)
    angles = np.arange(N, dtype=np.float32)[:, None] * freqs
    sin = np.sin(angles).astype(np.float32)
    cos = np.cos(angles).astype(np.float32)
    sin_i = np.stack([sin, sin], axis=-1).reshape(N, HD)
    cos_i = np.stack([cos, cos], axis=-1).reshape(N, HD)
    cs = np.concatenate([cos_i.T, cos_i.T], 0).astype(bf)  # [128, N]
    sn = np.concatenate([sin_i.T, sin_i.T], 0).astype(bf)

    R = np.zeros((HD, HD), np.float32)
    for d in range(32):
        R[d, 2 * d + 1] = -1.0
    for d in range(32, 64):
        R[d, 2 * (d - 32)] = 1.0
    R2 = np.zeros((128, 128), np.float32)
    R2[:64, :64] = R
    R2[64:, 64:] = R
    r2t = np.ascontiguousarray(R2.T).astype(bf)

    in_maps = []
    for c in range(N_CORES):
        g, hg = c // 4, c % 4
        heads = range(4 * hg, 4 * hg + 4)
        w_qk = np.concatenate(
            [np.concatenate([w_qkv[:, h * 64:(h + 1) * 64],
                             w_qkv[:, DIM + h * 64: DIM + (h + 1) * 64]], axis=1)
             for h in heads], axis=1)
        w_v = np.concatenate(
            [w_qkv[:, 2 * DIM + h * 64: 2 * DIM + (h + 1) * 64] for h in heads], axis=1)
        w_o = np.ascontiguousarray(w_out[4 * hg * 64:(4 * hg + 4) * 64, :])
        b_o = np.ascontiguousarray((b_out / 4.0).reshape(8, 128).T)
        in_maps.append({
            "x_t": np.ascontiguousarray(x[g].T).astype(bf),
            "w_qk": np.ascontiguousarray(w_qk).astype(bf),
            "w_v": np.ascontiguousarray(w_v).astype(bf),
            "w_o": w_o.astype(bf),
            "b_o": b_o.astype(np.float32),
            "cs": np.ascontiguousarray(cs),
            "sn": np.ascontiguousarray(sn),
            "r2t": r2t,
            "id128": np.eye(128, dtype=bf),
        })
    return in_maps


def build_nc(with_collective=True, dve_groups=(0, 3, 5), batch_w=True):
    import concourse.bass as bass  # noqa: F401
    import concourse.mybir as mybir
    import concourse.tile as tile
    from concourse import bacc

    f32 = mybir.dt.float32
    bf16 = mybir.dt.bfloat16
    i16 = mybir.dt.int16
    mult = mybir.AluOpType.mult
    add = mybir.AluOpType.add
    Exp = mybir.ActivationFunctionType.Exp

    # Schraudolph-style exp for the DVE offload path: e^(x/8) computed as
    # bf16 bit pattern round(x * 16*log2(e) + (127*128 - c)) written as
    # int16 and bitcast to bf16. c tuned for round-to-nearest; max rel err
    # ~3% which the softmax tolerates (weights err averages out across j).
    SCH_A = 128.0 * 1.4426950408889634 * 0.125
    SCH_B = 16256.0 - 5.7

    nc = bacc.Bacc("TRN2", target_bir_lowering=False, debug=False,
                   num_devices=N_CORES)
    x_t = nc.dram_tensor("x_t", [DIM, N], bf16, kind="ExternalInput")
    w_qk = nc.dram_tensor("w_qk", [DIM, 512], bf16, kind="ExternalInput")
    w_v = nc.dram_tensor("w_v", [DIM, 256], bf16, kind="ExternalInput")
    w_o = nc.dram_tensor("w_o", [256, DIM], bf16, kind="ExternalInput")
    b_o = nc.dram_tensor("b_o", [128, 8], f32, kind="ExternalInput")
    cs_d = nc.dram_tensor("cs", [128, N], bf16, kind="ExternalInput")
    sn_d = nc.dram_tensor("sn", [128, N], bf16, kind="ExternalInput")
    r2t_d = nc.dram_tensor("r2t", [128, 128], bf16, kind="ExternalInput")
    id_d = nc.dram_tensor("id128", [128, 128], bf16, kind="ExternalInput")
    # partials travel bf16: halves the rs_in store traffic and the
    # ReduceScatter payload (the f32-measured tail allowance stays a
    # strict upper bound); the host unshard converts back to f32
    y_out = nc.dram_tensor("y", [4, 256, 512], bf16, kind="ExternalOutput")

    with tile.TileContext(nc) as tc:
        with (
            tc.tile_pool(name="persist", bufs=1) as persist,
            tc.tile_pool(name="epool", bufs=5) as epool,
            tc.tile_pool(name="npool", bufs=2) as npool,
            tc.tile_pool(name="rpool", bufs=4) as rpool,
            tc.tile_pool(name="outp", bufs=4) as outp,
            tc.tile_pool(name="ppS", bufs=3, space="PSUM") as ppS,
            tc.tile_pool(name="ppA", bufs=2, space="PSUM") as ppA,
            tc.tile_pool(name="dram", bufs=8, space="DRAM") as dram,
        ):
            qp = persist.tile([64, 4, N], bf16)            # q'^T per head [d64, n]
            kp = persist.tile([64, 4, N], bf16)            # k'^T per head [d64, n]
            vsb = persist.tile([128, NJT, 4, 65], bf16)    # v + ones col per j-tile
            wo_sb = persist.tile([128, 2, DIM], bf16)
            b_sb = persist.tile([128, 8], f32)
            osb = persist.tile([128, 2, N], bf16)          # attn out, [d-kt, i]
            id_sb = persist.tile([128, 128], bf16)

            # ---- pair bookkeeping ---------------------------------------
            e_tiles = {}          # pair -> e_t tile [128, NJT, 512]
            emitted = {}          # pair -> set of emitted score groups
            nsb_tiles = {}        # iq -> [128, NIT, 256] bf16
            rs_tiles = {}         # iq -> rs_in dram tile

            def pq(P):
                return P // 4, P % 4  # (iq, h)

            # which exp groups run on DVE instead of ACT (empirically tuned:
            # the placement interacts with the ps_s ring recycling order)
            DVE_EXP_GROUPS = dve_groups

            def emit_scores_group(P, g):
                """Scores + exp for j-tiles 2g, 2g+1 of pair P."""
                if g in emitted.setdefault(P, set()):
                    return
                emitted[P].add(g)
                iq, h = pq(P)
                if P not in e_tiles:
                    e_tiles[P] = epool.tile([128, NJT, 512], bf16, name="e_t")
                ps = ppS.tile([128, 1024], f32, name="psS")
                for t in range(2):
                    jt = 2 * g + t
                    nc.tensor.matmul(
                        ps[:, t * 512:(t + 1) * 512],
                        lhsT=kp[:, h, jt * 128:(jt + 1) * 128],
                        rhs=qp[:, h, iq * 512:(iq + 1) * 512],
                        start=True, stop=True,
                    )
                e_out = e_tiles[P][:, 2 * g:2 * g + 2, :]
                # head-3 pairs offload EARLY groups instead: their late exps
                # gate the next quarter's P@V + transpose chain
                dve_set = (0, 3) if h == 3 else DVE_EXP_GROUPS
                if P >= 4 and g in dve_set:
                    # approximate exp on the otherwise-idle DVE to unload ACT
                    with nc.allow_low_precision(reason="schraudolph exp"):
                        nc.vector.tensor_scalar(
                            e_out.bitcast(i16), ps[:], SCH_A, SCH_B,
                            op0=mult, op1=add)
                else:
                    nc.scalar.activation(e_out, ps[:], Exp, scale=0.125)

            pv_ps = {}  # (P, it) -> psum accum tile
            pv_emitted = set()

            def emit_pv_half(P, it, hi):
                """8 accumulation matmuls for i-tile `it` of pair P over the
                low or high 8 j-tiles."""
                if (P, it, hi) in pv_emitted:
                    return
                pv_emitted.add((P, it, hi))
                iq, h = pq(P)
                e_t = e_tiles[P]
                if not hi:
                    pv_ps[(P, it)] = ppA.tile([128, 512], f32, name="psA")
                ps = pv_ps[(P, it)]
                for jj in range(8):
                    jt = 8 * hi + jj
                    nc.tensor.matmul(
                        ps[:, 0:65],
                        lhsT=e_t[:, jt, it * 128:(it + 1) * 128],
                        rhs=vsb[:, jt, h, :],
                        start=(jt == 0), stop=(jt == 15),
                    )

            def emit_normalize(P, it):
                iq, h = pq(P)
                ps = pv_ps.pop((P, it))
                if iq not in nsb_tiles:
                    nsb_tiles[iq] = npool.tile([128, NIT, 256], bf16, name="nsb")
                recip = rpool.tile([128, 1], f32, name="recip")
                with nc.allow_low_precision(reason="softmax denom recip"):
                    nc.vector.reciprocal(recip[:], ps[:, 64:65])
                nc.vector.tensor_scalar_mul(
                    nsb_tiles[iq][:, it, h * 64:(h + 1) * 64],
                    ps[:, 0:64], recip[:, 0:1])

            def emit_transpose(iq, it):
                # [128 i, 256 d] -> osb[:, kt, i] via DMA crossbar transpose
                nc.sync.dma_start_transpose(
                    out=osb[:, :, iq * 512 + it * 128: iq * 512 + (it + 1) * 128],
                    in_=nsb_tiles[iq][:, it, :],
                )

            def emit_transpose_pe(iq, it):
                # latency-critical tail variant: PE transpose + DVE copy
                # (~0.5us) instead of the ~2.5us DMA crossbar round trip
                ps_t = ppS.tile([128, 256], bf16, name="psS")
                for kt in range(2):
                    nc.tensor.transpose(
                        ps_t[:, kt * 128:(kt + 1) * 128],
                        nsb_tiles[iq][:, it, kt * 128:(kt + 1) * 128],
                        id_sb[:])
                nc.vector.tensor_copy(
                    osb[:, :, iq * 512 + it * 128: iq * 512 + (it + 1) * 128],
                    ps_t[:].rearrange("p (kt m) -> p kt m", kt=2))

            ot_pair = {}

            def emit_outproj_oc(iq, oc):
                if iq not in rs_tiles:
                    rs_tiles[iq] = dram.tile([1024, 512], bf16, name=f"rs_in_{iq}")
                ps = ppS.tile([128, 512], f32, name="psS")
                for kt in range(2):
                    nc.tensor.matmul(
                        ps[:, :],
                        lhsT=wo_sb[:, kt, oc * 128:(oc + 1) * 128],
                        rhs=osb[:, kt, iq * 512:(iq + 1) * 512],
                        start=(kt == 0), stop=(kt == 1),
                    )
                # PSUM is only engine-reachable (no DMA); split the drain+
                # bias across DVE and ACT (Identity takes a bias AP), and
                # batch two ocs per staging tile so the store descriptor
                # generation chain is 4 instructions, not 8
                if oc % 2 == 0:
                    ot_pair[iq] = outp.tile([128, 2, 512], bf16, name="o_t")
                o_t = ot_pair[iq]
                if oc % 2 == 0:
                    nc.vector.tensor_scalar_add(o_t[:, 0, :], ps[:, :], b_sb[:, oc:oc + 1])
                else:
                    nc.scalar.activation(o_t[:, 1, :], ps[:, :],
                                         mybir.ActivationFunctionType.Identity,
                                         bias=b_sb[:, oc:oc + 1])
                    nc.sync.dma_start(
                        rs_tiles[iq][(oc - 1) * 128:(oc + 1) * 128, :]
                        .rearrange("(a p) f -> p a f", a=2),
                        o_t[:])

            def rs_fire(iq):
                rs_in = rs_tiles[iq]
                if with_collective:
                    rs_out = dram.tile([256, 512], bf16, name=f"rs_out_{iq}")
                    nc.gpsimd.collective_compute(
                        "ReduceScatter",
                        mybir.AluOpType.add,
                        replica_groups=GROUPS,
                        ins=[rs_in[:]],
                        outs=[rs_out[:]],
                    )
                    nc.sync.dma_start(y_out[iq], rs_out[:])
                else:
                    nc.sync.dma_start(y_out[iq], rs_in[0:256, :])

            # PV emission order within a drive: (it, hi) per g slot
            PV_ORDER = [(0, 0), (1, 0), (0, 1), (1, 1), (2, 0), (3, 0), (2, 1), (3, 1)]

            def drive(P):
                """Emit one pipeline step: pair P+1's remaining scores
                interleaved with pair P-1's P@V, plus carried transpose /
                out-projection / ReduceScatter work."""
                iq, h = pq(P)
                for g in range(8):
                    if P - 1 >= 0:
                        it, hi = PV_ORDER[g]
                        emit_pv_half(P - 1, it, hi)
                        if hi:
                            emit_normalize(P - 1, it)
                            iqp, hp = pq(P - 1)
                            if hp == 3:
                                emit_transpose(iqp, it)
                                if it == NIT - 1:
                                    # e_t slot of P-1 and nsb of iqp retire here
                                    del e_tiles[P - 1]
                                    del nsb_tiles[iqp]
                    if 0 <= P + 1 < NPAIR:
                        emit_scores_group(P + 1, g)
                    elif P == NPAIR - 1 and g >= 4:
                        # last drive has no next-pair scores: pull the final
                        # pair's P@V forward out of the tail (low halves for
                        # all i-tiles, then hi+normalize+transpose for the
                        # first two)
                        emit_pv_half(P, g - 4, 0)
                    # out-projection of quarter iq-1 spread across two drives
                    if iq >= 1 and h in (1, 2) and g % 2 == 1:
                        emit_outproj_oc(iq - 1, 4 * (h - 1) + g // 2)
                        if h == 2 and g == 7:
                            rs_fire(iq - 1)

            # ---------------- Phase 1: QKV projection + RoPE ----------------
            with (
                tc.tile_pool(name="xw", bufs=1) as xw,
                tc.tile_pool(name="scr", bufs=4) as scr,
            ):
                cs_sb = xw.tile([128, N], bf16)
                sn_sb = xw.tile([128, N], bf16)
                r2t_sb = xw.tile([128, 128], bf16)
                wqk = xw.tile([128, 8, 512], bf16)
                wv = xw.tile([128, 8, 256], bf16)
                xt0 = xw.tile([128, 8, 512], bf16, name="xt", bufs=2)
                # first compute chunk (wqk/x kt=0) leads both queues so the
                # first qk matmul starts as early as possible; remaining
                # weights ride single multi-descriptor DMAs so the SWDGE
                # queue's ~1us per-instruction generation cost doesn't starve
                # the rope (cs/sn) and v-projection (wv) mid-phase-1
                nc.gpsimd.dma_start(wqk[:, 0, :], w_qk[0:128, :])
                nc.sync.dma_start(xt0[:, 0, :], x_t[0:128, 0:512])
                if batch_w:
                    nc.gpsimd.dma_start(
                        wqk[:, 1:4, :],
                        w_qk[128:512, :].rearrange("(kt p) d -> p kt d", p=128))
                    nc.gpsimd.dma_start(
                        wqk[:, 4:8, :],
                        w_qk[512:1024, :].rearrange("(kt p) d -> p kt d", p=128))
                    nc.gpsimd.dma_start(
                        wv[:, :, :],
                        w_v.ap().rearrange("(kt p) d -> p kt d", p=128))
                else:
                    for kt in range(1, 8):
                        nc.gpsimd.dma_start(wqk[:, kt, :], w_qk[kt * 128:(kt + 1) * 128, :])
                    for kt in range(8):
                        nc.gpsimd.dma_start(wv[:, kt, :], w_v[kt * 128:(kt + 1) * 128, :])
                for kt in range(1, 8):
                    nc.sync.dma_start(
                        xt0[:, kt, :], x_t[kt * 128:(kt + 1) * 128, 0:512])
                nc.sync.dma_start(r2t_sb[:], r2t_d.ap())
                nc.sync.dma_start(cs_sb[:, 0:512], cs_d[:, 0:512])
                nc.sync.dma_start(sn_sb[:, 0:512], sn_d[:, 0:512])
                nc.gpsimd.dma_start(id_sb[:], id_d.ap())
                nc.vector.memset(vsb[:, :, :, 64:65], 1.0)
                nc.gpsimd.dma_start(b_sb[:], b_o.ap())
                nc.gpsimd.dma_start(
                    wo_sb[:, :, :],
                    w_o.ap().rearrange("(kt p) d -> p kt d", p=128))

                for ic4 in range(4):
                    # early score/exp groups interleaved at 8 points per ic4
                    # block (after each head's rope and each v-proj) so ACT
                    # stays busy without serializing the PE on the ps_s ring:
                    # ic4 c emits groups 2(c-1), 2(c-1)+1 for pairs 0..3,
                    # whose j-tiles were produced by block c-1.
                    if ic4 >= 1:
                        gs = (2 * (ic4 - 1), 2 * (ic4 - 1) + 1)
                        pend_scores = [(Pp, g) for g in gs for Pp in (0, 1, 2)]
                        pend_scores += [(3, g) for g in gs]
                    else:
                        pend_scores = []

                    def pump_scores():
                        if pend_scores:
                            emit_scores_group(*pend_scores.pop(0))

                    isl = slice(ic4 * 512, (ic4 + 1) * 512)
                    if ic4 == 0:
                        xt = xt0
                    else:
                        xt = xw.tile([128, 8, 512], bf16, name="xt", bufs=2)
                        for kt in range(8):
                            nc.sync.dma_start(xt[:, kt, :], x_t[kt * 128:(kt + 1) * 128, isl])
                        nc.sync.dma_start(cs_sb[:, isl], cs_d[:, isl])
                        nc.sync.dma_start(sn_sb[:, isl], sn_d[:, isl])
                    # rope chain for head h-1 emitted after head h's qk matmuls
                    # so the rot matmul never heads the PE queue waiting on its
                    # DVE copy round trip
                    pend = None

                    def rope_chain(h, qks):
                        ps_rot = ppS.tile([128, 512], f32, name="psS")
                        nc.tensor.matmul(ps_rot[:, 0:512], lhsT=r2t_sb[:],
                                         rhs=qks[:], start=True, stop=True)
                        t1 = scr.tile([128, 512], bf16, name="t1")
                        nc.vector.tensor_tensor(t1[:], qks[:], cs_sb[:, isl], op=mult)
                        t2 = scr.tile([128, 512], bf16, name="t2")
                        nc.vector.tensor_tensor(t2[:], ps_rot[:, 0:512], sn_sb[:, isl], op=mult)
                        nc.vector.tensor_tensor(qp[:, h, isl], t1[0:64, :], t2[0:64, :], op=add)
                        nc.vector.tensor_tensor(kp[:, h, isl], t1[64:128, :], t2[64:128, :], op=add)

                    for h in range(4):
                        ps_qk = ppA.tile([128, 512], f32, name="psA")
                        for kt in range(8):
                            nc.tensor.matmul(
                                ps_qk[:, 0:512],
                                lhsT=wqk[:, kt, h * 128:(h + 1) * 128],
                                rhs=xt[:, kt, :],
                                start=(kt == 0), stop=(kt == 7),
                            )
                        qks = scr.tile([128, 512], bf16, name="qks")
                        # ACT has phase-1 slack; DVE is co-saturated there
                        nc.scalar.copy(qks[:], ps_qk[:, 0:512])
                        if pend is not None:
                            rope_chain(*pend)
                            pump_scores()
                        pend = (h, qks)
                    rope_chain(*pend)
                    pump_scores()
                    for it2 in range(4):
                        it = ic4 * 4 + it2
                        ps_v = ppA.tile([128, 512], f32, name="psA")
                        for kt in range(8):
                            nc.tensor.matmul(
                                ps_v[:, 0:256],
                                lhsT=xt[:, kt, it2 * 128:(it2 + 1) * 128],
                                rhs=wv[:, kt, :],
                                start=(kt == 0), stop=(kt == 7),
                            )
                        nc.vector.tensor_copy(
                            vsb[:, it, :, 0:64],
                            ps_v[:, 0:256].rearrange("p (h d) -> p h d", d=64),
                        )
                        pump_scores()
                    while pend_scores:
                        pump_scores()

            # ---------------- Phase 2: pipelined attention ----------------
            # phase 1 covered groups 0..5 for pairs 0..3; pair 0's last two
            # groups have no preceding drive, so emit them first
            for g in range(6, 8):
                emit_scores_group(0, g)
            for P in range(NPAIR):
                drive(P)
            # tail: PV of the last pair + low-latency PE transposes, then the
            # final out-projection + ReduceScatter chunk
            for g in range(8):
                it, hi = PV_ORDER[g]
                emit_pv_half(NPAIR - 1, it, hi)
                if hi:
                    emit_normalize(NPAIR - 1, it)
                    emit_transpose_pe(NIQ - 1, it)

            for oc in range(8):
                emit_outproj_oc(NIQ - 1, oc)
            rs_fire(NIQ - 1)

    nc.compile()
    return nc


def _get_nc():
    if "nc" not in _COMPILED:
        _COMPILED["nc"] = build_nc()
    return _COMPILED["nc"]


def kernel(x, w_qkv, w_out, b_out):
    from concourse import bass_utils

    x = np.asarray(x, dtype=np.float32)
    w_qkv = np.asarray(w_qkv, dtype=np.float32)
    w_out = np.asarray(w_out, dtype=np.float32)
    b_out = np.asarray(b_out, dtype=np.float32)

    nc = _get_nc()
    in_maps = _host_prep(x, w_qkv, w_out, b_out)
    res = bass_utils.run_bass_kernel_spmd(nc, in_maps, list(range(N_CORES)))

    out = np.zeros((B, N, DIM), np.float32)
    for c in range(N_CORES):
        g, pos = c // 4, c % 4
        y = np.asarray(res.results[c]["y"], dtype=np.float32)  # [4, 256, 512]
        for ib in range(4):
            out[g, ib * 512:(ib + 1) * 512, pos * 256:(pos + 1) * 256] = y[ib].T
    return out


if __name__ == "__main__":
    rng = np.random.default_rng(0)
    x = rng.standard_normal((B, N, DIM)).astype(np.float32)
    w_qkv = (rng.standard_normal((DIM, 3 * DIM)) * DIM ** -0.5).astype(np.float32)
    w_out = (rng.standard_normal((DIM, DIM)) * DIM ** -0.5).astype(np.float32)
    b_out = np.zeros(DIM, np.float32)
    out = kernel(x, w_qkv, w_out, b_out)
    print("out", out.shape, out.dtype, float(np.abs(out).max()))


# revision 73
# speedup vs baseline: 1.3963x; 1.0008x over previous
"""Multi-head attention with RoPE on 8 Trainium2 NeuronCores.

Sharding: core c -> (batch g = c//4, head-group hg = c%4 of 4 heads).

Per core, all-bf16 dataflow (PSUM accumulation in f32):
  - QKV projection from column slices of w_qkv; RoPE via a signed-permutation
    matmul + DVE combines, q'/k' stored dim-major [64, h, n] in bf16.
  - Attention runs per (i-quarter, head) pair: scores S^T = K'_jt Q'_iq as
    [128 j x 512 i] PSUM tiles, exp on ACT (no max-subtraction: scores O(1)
    by construction) into bf16 e_t tiles.
  - P@V uses the swapped operand order: lhsT = e_t i-slice (stationary),
    rhs = V j-tile with a ones column appended, so the output lands [i, d+1]
    with full 128-partition utilization (65 PE rows per j-tile instead of
    512) and the softmax denominator in column 64.
  - Normalize on DVE via per-partition reciprocal * tensor_scalar, then the
    [i, d] -> [d, i] flip runs on the DMA crossbar transpose engine (free on
    PE), feeding the out-projection (rows of w_out for the local heads).
  - Partials summed across each 4-core batch group with chunked
    ReduceScatter (one per 512-row i-quarter, overlapped with compute);
    each core lands a distinct [256-outcol x 512-row] quarter per chunk.

The emission schedule pipelines pairs: during pair P's score/exp stream the
PE also runs pair P-1's P@V, the previous i-quarter's transposes and the
out-projection + ReduceScatter of the quarter before that. Roughly a third
of the exp groups run on the otherwise-idle DVE via a Schraudolph-style
approximate exp (tensor_scalar to int16 bf16-bit-pattern), keeping ACT (the
exp engine, the secondary bottleneck after PE) off the critical path.
"""

import numpy as np

H, HD = 16, 64
B, N, DIM = 2, 2048, 1024
N_CORES = 8
GROUPS = [[0, 1, 2, 3], [4, 5, 6, 7]]

NJT = 16          # 128-token j tiles
NIQ = 4           # 512-token i quarters (== ReduceScatter chunks)
NIT = 4           # 128-token i tiles per quarter
NPAIR = NIQ * 4   # (i-quarter, head) pairs per core

_COMPILED = {}


def _host_prep(x, w_qkv, w_out, b_out):
    import ml_dtypes

    bf = ml_dtypes.bfloat16
    freqs = 10000.0 ** (-np.arange(0, HD, 2, dtype=np.float32) / HD)
    angles = np.arange(N, dtype=np.float32)[:, None] * freqs
    sin = np.sin(angles).astype(np.float32)
    cos = np.cos(angles).astype(np.float32)
    sin_i = np.stack([sin, sin], axis=-1).reshape(N, HD)
    cos_i = np.stack([cos, cos], axis=-1).reshape(N, HD)
    cs = np.concatenate([cos_i.T, cos_i.T], 0).astype(bf)  # [128, N]
    sn = np.concatenate([sin_i.T, sin_i.T], 0).astype(bf)

    R = np.zeros((HD, HD), np.float32)
    for d in range(32):
        R[d, 2 * d + 1] = -1.0
    for d in range(32, 64):
        R[d, 2 * (d - 32)] = 1.0
    R2 = np.zeros((128, 128), np.float32)
    R2[:64, :64] = R
    R2[64:, 64:] = R
    r2t = np.ascontiguousarray(R2.T).astype(bf)

    in_maps = []
    for c in range(N_CORES):
        g, hg = c // 4, c % 4
        heads = range(4 * hg, 4 * hg + 4)
        w_qk = np.concatenate(
            [np.concatenate([w_qkv[:, h * 64:(h + 1) * 64],
                             w_qkv[:, DIM + h * 64: DIM + (h + 1) * 64]], axis=1)
             for h in heads], axis=1)
        w_v = np.concatenate(
            [w_qkv[:, 2 * DIM + h * 64: 2 * DIM + (h + 1) * 64] for h in heads], axis=1)
        w_o = np.ascontiguousarray(w_out[4 * hg * 64:(4 * hg + 4) * 64, :])
        b_o = np.ascontiguousarray((b_out / 4.0).reshape(8, 128).T)
        in_maps.append({
            "x_t": np.ascontiguousarray(x[g].T).astype(bf),
            "w_qk": np.ascontiguousarray(w_qk).astype(bf),
            "w_v": np.ascontiguousarray(w_v).astype(bf),
            "w_o": w_o.astype(bf),
            "b_o": b_o.astype(np.float32),
            "cs": np.ascontiguousarray(cs),
            "sn": np.ascontiguousarray(sn),
            "r2t": r2t,
            "id128": np.eye(128, dtype=bf),
        })
    return in_maps


def build_nc(with_collective=True, dve_groups=(0, 3, 5), batch_w=True):
    import concourse.bass as bass  # noqa: F401
    import concourse.mybir as mybir
    import concourse.tile as tile
    from concourse import bacc

    f32 = mybir.dt.float32
    bf16 = mybir.dt.bfloat16
    i16 = mybir.dt.int16
    mult = mybir.AluOpType.mult
    add = mybir.AluOpType.add
    Exp = mybir.ActivationFunctionType.Exp

    # Schraudolph-style exp for the DVE offload path: e^(x/8) computed as
    # bf16 bit pattern round(x * 16*log2(e) + (127*128 - c)) written as
    # int16 and bitcast to bf16. c tuned for round-to-nearest; max rel err
    # ~3% which the softmax tolerates (weights err averages out across j).
    SCH_A = 128.0 * 1.4426950408889634 * 0.125
    SCH_B = 16256.0 - 5.7

    nc = bacc.Bacc("TRN2", target_bir_lowering=False, debug=False,
                   num_devices=N_CORES)
    x_t = nc.dram_tensor("x_t", [DIM, N], bf16, kind="ExternalInput")
    w_qk = nc.dram_tensor("w_qk", [DIM, 512], bf16, kind="ExternalInput")
    w_v = nc.dram_tensor("w_v", [DIM, 256], bf16, kind="ExternalInput")
    w_o = nc.dram_tensor("w_o", [256, DIM], bf16, kind="ExternalInput")
    b_o = nc.dram_tensor("b_o", [128, 8], f32, kind="ExternalInput")
    cs_d = nc.dram_tensor("cs", [128, N], bf16, kind="ExternalInput")
    sn_d = nc.dram_tensor("sn", [128, N], bf16, kind="ExternalInput")
    r2t_d = nc.dram_tensor("r2t", [128, 128], bf16, kind="ExternalInput")
    id_d = nc.dram_tensor("id128", [128, 128], bf16, kind="ExternalInput")
    # partials travel bf16: halves the rs_in store traffic and the
    # ReduceScatter payload (the f32-measured tail allowance stays a
    # strict upper bound); the host unshard converts back to f32
    y_out = nc.dram_tensor("y", [4, 256, 512], bf16, kind="ExternalOutput")

    with tile.TileContext(nc) as tc:
        with (
            tc.tile_pool(name="persist", bufs=1) as persist,
            tc.tile_pool(name="epool", bufs=5) as epool,
            tc.tile_pool(name="npool", bufs=2) as npool,
            tc.tile_pool(name="rpool", bufs=4) as rpool,
            tc.tile_pool(name="outp", bufs=4) as outp,
            tc.tile_pool(name="ppS", bufs=3, space="PSUM") as ppS,
            tc.tile_pool(name="ppA", bufs=2, space="PSUM") as ppA,
            tc.tile_pool(name="dram", bufs=8, space="DRAM") as dram,
        ):
            qp = persist.tile([64, 4, N], bf16)            # q'^T per head [d64, n]
            kp = persist.tile([64, 4, N], bf16)            # k'^T per head [d64, n]
            vsb = persist.tile([128, NJT, 4, 65], bf16)    # v + ones col per j-tile
            wo_sb = persist.tile([128, 2, DIM], bf16)
            b_sb = persist.tile([128, 8], f32)
            osb = persist.tile([128, 2, N], bf16)          # attn out, [d-kt, i]
            id_sb = persist.tile([128, 128], bf16)

            # ---- pair bookkeeping ---------------------------------------
            e_tiles = {}          # pair -> e_t tile [128, NJT, 512]
            emitted = {}          # pair -> set of emitted score groups
            nsb_tiles = {}        # iq -> [128, NIT, 256] bf16
            rs_tiles = {}         # iq -> rs_in dram tile

            def pq(P):
                return P // 4, P % 4  # (iq, h)

            # which exp groups run on DVE instead of ACT (empirically tuned:
            # the placement interacts with the ps_s ring recycling order)
            DVE_EXP_GROUPS = dve_groups

            def emit_scores_group(P, g):
                """Scores + exp for j-tiles 2g, 2g+1 of pair P."""
                if g in emitted.setdefault(P, set()):
                    return
                emitted[P].add(g)
                iq, h = pq(P)
                if P not in e_tiles:
                    e_tiles[P] = epool.tile([128, NJT, 512], bf16, name="e_t")
                ps = ppS.tile([128, 1024], f32, name="psS")
                for t in range(2):
                    jt = 2 * g + t
                    nc.tensor.matmul(
                        ps[:, t * 512:(t + 1) * 512],
                        lhsT=kp[:, h, jt * 128:(jt + 1) * 128],
                        rhs=qp[:, h, iq * 512:(iq + 1) * 512],
                        start=True, stop=True,
                    )
                e_out = e_tiles[P][:, 2 * g:2 * g + 2, :]
                # head-3 pairs offload EARLY groups instead: their late exps
                # gate the next quarter's P@V + transpose chain
                dve_set = (0, 3) if h == 3 else DVE_EXP_GROUPS
                if P >= 4 and g in dve_set:
                    # approximate exp on the otherwise-idle DVE to unload ACT
                    with nc.allow_low_precision(reason="schraudolph exp"):
                        nc.vector.tensor_scalar(
                            e_out.bitcast(i16), ps[:], SCH_A, SCH_B,
                            op0=mult, op1=add)
                else:
                    nc.scalar.activation(e_out, ps[:], Exp, scale=0.125)

            pv_ps = {}  # (P, it) -> psum accum tile
            pv_emitted = set()

            def emit_pv_half(P, it, hi):
                """8 accumulation matmuls for i-tile `it` of pair P over the
                low or high 8 j-tiles."""
                if (P, it, hi) in pv_emitted:
                    return
                pv_emitted.add((P, it, hi))
                iq, h = pq(P)
                e_t = e_tiles[P]
                if not hi:
                    pv_ps[(P, it)] = ppA.tile([128, 512], f32, name="psA")
                ps = pv_ps[(P, it)]
                for jj in range(8):
                    jt = 8 * hi + jj
                    nc.tensor.matmul(
                        ps[:, 0:65],
                        lhsT=e_t[:, jt, it * 128:(it + 1) * 128],
                        rhs=vsb[:, jt, h, :],
                        start=(jt == 0), stop=(jt == 15),
                    )

            def emit_normalize(P, it):
                iq, h = pq(P)
                ps = pv_ps.pop((P, it))
                if iq not in nsb_tiles:
                    nsb_tiles[iq] = npool.tile([128, NIT, 256], bf16, name="nsb")
                recip = rpool.tile([128, 1], f32, name="recip")
                with nc.allow_low_precision(reason="softmax denom recip"):
                    nc.vector.reciprocal(recip[:], ps[:, 64:65])
                nc.vector.tensor_scalar_mul(
                    nsb_tiles[iq][:, it, h * 64:(h + 1) * 64],
                    ps[:, 0:64], recip[:, 0:1])

            def emit_transpose(iq, it):
                # [128 i, 256 d] -> osb[:, kt, i] via DMA crossbar transpose
                nc.sync.dma_start_transpose(
                    out=osb[:, :, iq * 512 + it * 128: iq * 512 + (it + 1) * 128],
                    in_=nsb_tiles[iq][:, it, :],
                )

            def emit_transpose_pe(iq, it):
                # latency-critical tail variant: PE transpose + DVE copy
                # (~0.5us) instead of the ~2.5us DMA crossbar round trip
                ps_t = ppS.tile([128, 256], bf16, name="psS")
                for kt in range(2):
                    nc.tensor.transpose(
                        ps_t[:, kt * 128:(kt + 1) * 128],
                        nsb_tiles[iq][:, it, kt * 128:(kt + 1) * 128],
                        id_sb[:])
                nc.vector.tensor_copy(
                    osb[:, :, iq * 512 + it * 128: iq * 512 + (it + 1) * 128],
                    ps_t[:].rearrange("p (kt m) -> p kt m", kt=2))

            ot_pair = {}

            def emit_outproj_oc(iq, oc):
                if iq not in rs_tiles:
                    rs_tiles[iq] = dram.tile([1024, 512], bf16, name=f"rs_in_{iq}")
                ps = ppS.tile([128, 512], f32, name="psS")
                for kt in range(2):
                    nc.tensor.matmul(
                        ps[:, :],
                        lhsT=wo_sb[:, kt, oc * 128:(oc + 1) * 128],
                        rhs=osb[:, kt, iq * 512:(iq + 1) * 512],
                        start=(kt == 0), stop=(kt == 1),
                    )
                # PSUM is only engine-reachable (no DMA); split the drain+
                # bias across DVE and ACT (Identity takes a bias AP), and
                # batch two ocs per staging tile so the store descriptor
                # generation chain is 4 instructions, not 8
                if oc % 2 == 0:
                    ot_pair[iq] = outp.tile([128, 2, 512], bf16, name="o_t")
                o_t = ot_pair[iq]
                if oc % 2 == 0:
                    nc.vector.tensor_scalar_add(o_t[:, 0, :], ps[:, :], b_sb[:, oc:oc + 1])
                else:
                    nc.scalar.activation(o_t[:, 1, :], ps[:, :],
                                         mybir.ActivationFunctionType.Identity,
                                         bias=b_sb[:, oc:oc + 1])
                    nc.sync.dma_start(
                        rs_tiles[iq][(oc - 1) * 128:(oc + 1) * 128, :]
                        .rearrange("(a p) f -> p a f", a=2),
                        o_t[:])

            def rs_fire(iq):
                rs_in = rs_tiles[iq]
                if with_collective:
                    rs_out = dram.tile([256, 512], bf16, name=f"rs_out_{iq}")
                    nc.gpsimd.collective_compute(
                        "ReduceScatter",
                        mybir.AluOpType.add,
                        replica_groups=GROUPS,
                        ins=[rs_in[:]],
                        outs=[rs_out[:]],
                    )
                    nc.sync.dma_start(y_out[iq], rs_out[:])
                else:
                    nc.sync.dma_start(y_out[iq], rs_in[0:256, :])

            # PV emission order within a drive: (it, hi) per g slot
            PV_ORDER = [(0, 0), (1, 0), (0, 1), (1, 1), (2, 0), (3, 0), (2, 1), (3, 1)]

            def drive(P):
                """Emit one pipeline step: pair P+1's remaining scores
                interleaved with pair P-1's P@V, plus carried transpose /
                out-projection / ReduceScatter work."""
                iq, h = pq(P)
                for g in range(8):
                    if P - 1 >= 0:
                        it, hi = PV_ORDER[g]
                        emit_pv_half(P - 1, it, hi)
                        if hi:
                            emit_normalize(P - 1, it)
                            iqp, hp = pq(P - 1)
                            if hp == 3:
                                emit_transpose(iqp, it)
                                if it == NIT - 1:
                                    # e_t slot of P-1 and nsb of iqp retire here
                                    del e_tiles[P - 1]
                                    del nsb_tiles[iqp]
                    if 0 <= P + 1 < NPAIR:
                        emit_scores_group(P + 1, g)
                    elif P == NPAIR - 1 and g >= 4:
                        # last drive has no next-pair scores: pull the final
                        # pair's P@V forward out of the tail (low halves for
                        # all i-tiles, then hi+normalize+transpose for the
                        # first two)
                        emit_pv_half(P, g - 4, 0)
                    # out-projection of quarter iq-1 spread across two drives
                    if iq >= 1 and h in (1, 2) and g % 2 == 1:
                        emit_outproj_oc(iq - 1, 4 * (h - 1) + g // 2)
                        if h == 2 and g == 7:
                            rs_fire(iq - 1)

            # ---------------- Phase 1: QKV projection + RoPE ----------------
            with (
                tc.tile_pool(name="xw", bufs=1) as xw,
                tc.tile_pool(name="scr", bufs=4) as scr,
            ):
                cs_sb = xw.tile([128, N], bf16)
                sn_sb = xw.tile([128, N], bf16)
                r2t_sb = xw.tile([128, 128], bf16)
                wqk = xw.tile([128, 8, 512], bf16)
                wv = xw.tile([128, 8, 256], bf16)
                xt0 = xw.tile([128, 8, 512], bf16, name="xt", bufs=2)
                # first compute chunk (wqk/x kt=0) leads both queues so the
                # first qk matmul starts as early as possible; remaining
                # weights ride single multi-descriptor DMAs so the SWDGE
                # queue's ~1us per-instruction generation cost doesn't starve
                # the rope (cs/sn) and v-projection (wv) mid-phase-1
                nc.gpsimd.dma_start(wqk[:, 0, :], w_qk[0:128, :])
                nc.sync.dma_start(xt0[:, 0, :], x_t[0:128, 0:512])
                if batch_w:
                    nc.gpsimd.dma_start(
                        wqk[:, 1:4, :],
                        w_qk[128:512, :].rearrange("(kt p) d -> p kt d", p=128))
                    nc.gpsimd.dma_start(
                        wqk[:, 4:8, :],
                        w_qk[512:1024, :].rearrange("(kt p) d -> p kt d", p=128))
                    nc.gpsimd.dma_start(
                        wv[:, :, :],
                        w_v.ap().rearrange("(kt p) d -> p kt d", p=128))
                else:
                    for kt in range(1, 8):
                        nc.gpsimd.dma_start(wqk[:, kt, :], w_qk[kt * 128:(kt + 1) * 128, :])
                    for kt in range(8):
                        nc.gpsimd.dma_start(wv[:, kt, :], w_v[kt * 128:(kt + 1) * 128, :])
                for kt in range(1, 8):
                    nc.sync.dma_start(
                        xt0[:, kt, :], x_t[kt * 128:(kt + 1) * 128, 0:512])
                nc.sync.dma_start(r2t_sb[:], r2t_d.ap())
                nc.sync.dma_start(cs_sb[:, 0:512], cs_d[:, 0:512])
                nc.sync.dma_start(sn_sb[:, 0:512], sn_d[:, 0:512])
                nc.gpsimd.dma_start(id_sb[:], id_d.ap())
                nc.vector.memset(vsb[:, :, :, 64:65], 1.0)
                nc.gpsimd.dma_start(b_sb[:], b_o.ap())
                nc.gpsimd.dma_start(
                    wo_sb[:, :, :],
                    w_o.ap().rearrange("(kt p) d -> p kt d", p=128))

                for ic4 in range(4):
                    # early score/exp groups interleaved at 8 points per ic4
                    # block (after each head's rope and each v-proj) so ACT
                    # stays busy without serializing the PE on the ps_s ring:
                    # ic4 c emits groups 2(c-1), 2(c-1)+1 for pairs 0..3,
                    # whose j-tiles were produced by block c-1.
                    if ic4 >= 1:
                        gs = (2 * (ic4 - 1), 2 * (ic4 - 1) + 1)
                        pend_scores = [(Pp, g) for g in gs for Pp in (0, 1, 2)]
                        pend_scores += [(3, g) for g in gs]
                    else:
                        pend_scores = []

                    def pump_scores():
                        if pend_scores:
                            emit_scores_group(*pend_scores.pop(0))

                    isl = slice(ic4 * 512, (ic4 + 1) * 512)
                    if ic4 == 0:
                        xt = xt0
                    else:
                        xt = xw.tile([128, 8, 512], bf16, name="xt", bufs=2)
                        # one multi-descriptor DMA per block: 8x fewer HWDGE
                        # generator slots, which the transposes and stores
                        # also contend for
                        nc.sync.dma_start(
                            xt[:, :, :],
                            x_t[:, isl].rearrange("(kt p) f -> p kt f", p=128))
                        nc.sync.dma_start(cs_sb[:, isl], cs_d[:, isl])
                        nc.sync.dma_start(sn_sb[:, isl], sn_d[:, isl])
                    # rope chain for head h-1 emitted after head h's qk matmuls
                    # so the rot matmul never heads the PE queue waiting on its
                    # DVE copy round trip
                    pend = None

                    def rope_chain(h, qks):
                        ps_rot = ppS.tile([128, 512], f32, name="psS")
                        nc.tensor.matmul(ps_rot[:, 0:512], lhsT=r2t_sb[:],
                                         rhs=qks[:], start=True, stop=True)
                        t1 = scr.tile([128, 512], bf16, name="t1")
                        nc.vector.tensor_tensor(t1[:], qks[:], cs_sb[:, isl], op=mult)
                        t2 = scr.tile([128, 512], bf16, name="t2")
                        nc.vector.tensor_tensor(t2[:], ps_rot[:, 0:512], sn_sb[:, isl], op=mult)
                        nc.vector.tensor_tensor(qp[:, h, isl], t1[0:64, :], t2[0:64, :], op=add)
                        nc.vector.tensor_tensor(kp[:, h, isl], t1[64:128, :], t2[64:128, :], op=add)

                    for h in range(4):
                        ps_qk = ppA.tile([128, 512], f32, name="psA")
                        for kt in range(8):
                            nc.tensor.matmul(
                                ps_qk[:, 0:512],
                                lhsT=wqk[:, kt, h * 128:(h + 1) * 128],
                                rhs=xt[:, kt, :],
                                start=(kt == 0), stop=(kt == 7),
                            )
                        qks = scr.tile([128, 512], bf16, name="qks")
                        # ACT has phase-1 slack; DVE is co-saturated there
                        nc.scalar.copy(qks[:], ps_qk[:, 0:512])
                        if pend is not None:
                            rope_chain(*pend)
                            pump_scores()
                        pend = (h, qks)
                    rope_chain(*pend)
                    pump_scores()
                    for it2 in range(4):
                        it = ic4 * 4 + it2
                        ps_v = ppA.tile([128, 512], f32, name="psA")
                        for kt in range(8):
                            nc.tensor.matmul(
                                ps_v[:, 0:256],
                                lhsT=xt[:, kt, it2 * 128:(it2 + 1) * 128],
                                rhs=wv[:, kt, :],
                                start=(kt == 0), stop=(kt == 7),
                            )
                        nc.vector.tensor_copy(
                            vsb[:, it, :, 0:64],
                            ps_v[:, 0:256].rearrange("p (h d) -> p h d", d=64),
                        )
                        pump_scores()
                    while pend_scores:
                        pump_scores()

            # ---------------- Phase 2: pipelined attention ----------------
            # phase 1 covered groups 0..5 for pairs 0..3; pair 0's last two
            # groups have no preceding drive, so emit them first
            for g in range(6, 8):
                emit_scores_group(0, g)
            for P in range(NPAIR):
                drive(P)
            # tail: PV of the last pair + low-latency PE transposes, then the
            # final out-projection + ReduceScatter chunk
            for g in range(8):
                it, hi = PV_ORDER[g]
                emit_pv_half(NPAIR - 1, it, hi)
                if hi:
                    emit_normalize(NPAIR - 1, it)
                    emit_transpose_pe(NIQ - 1, it)

            for oc in range(8):
                emit_outproj_oc(NIQ - 1, oc)
            rs_fire(NIQ - 1)

    nc.compile()
    return nc


def _get_nc():
    if "nc" not in _COMPILED:
        _COMPILED["nc"] = build_nc()
    return _COMPILED["nc"]


def kernel(x, w_qkv, w_out, b_out):
    from concourse import bass_utils

    x = np.asarray(x, dtype=np.float32)
    w_qkv = np.asarray(w_qkv, dtype=np.float32)
    w_out = np.asarray(w_out, dtype=np.float32)
    b_out = np.asarray(b_out, dtype=np.float32)

    nc = _get_nc()
    in_maps = _host_prep(x, w_qkv, w_out, b_out)
    res = bass_utils.run_bass_kernel_spmd(nc, in_maps, list(range(N_CORES)))

    out = np.zeros((B, N, DIM), np.float32)
    for c in range(N_CORES):
        g, pos = c // 4, c % 4
        y = np.asarray(res.results[c]["y"], dtype=np.float32)  # [4, 256, 512]
        for ib in range(4):
            out[g, ib * 512:(ib + 1) * 512, pos * 256:(pos + 1) * 256] = y[ib].T
    return out


if __name__ == "__main__":
    rng = np.random.default_rng(0)
    x = rng.standard_normal((B, N, DIM)).astype(np.float32)
    w_qkv = (rng.standard_normal((DIM, 3 * DIM)) * DIM ** -0.5).astype(np.float32)
    w_out = (rng.standard_normal((DIM, DIM)) * DIM ** -0.5).astype(np.float32)
    b_out = np.zeros(DIM, np.float32)
    out = kernel(x, w_qkv, w_out, b_out)
    print("out", out.shape, out.dtype, float(np.abs(out).max()))
